# revision 4
# baseline (speedup 1.0000x reference)
"""ArcFace loss kernel for 8 Trainium2 NeuronCores.

Reference computation (per row i of cls_score [4096, 10000], label [4096]):
    tgt       = cls_score[i, label[i]]
    t         = clip(tgt, -1+eps, 1-eps)
    numerator = S * cos(acos(t) + M)            # == S*(t*cosM - sqrt(1-t^2)*sinM)
    excl      = sum_c exp(S*cls_score[i,c]) - exp(S*tgt)
    denom     = exp(numerator) + excl
    L_i       = numerator - log(denom)
    loss      = -mean(L_i)

Sharding: data-parallel over the batch dim, 512 rows per core. Each core
computes sum_i(L_i) * (-1/4096) for its shard; the partial scalars are summed
across cores at the end.
"""

import sys

sys.path.insert(0, "/opt/trn_rl_repo")

from contextlib import ExitStack

import numpy as np

import concourse.bass as bass
import concourse.tile as tile
from concourse import bacc, mybir
from concourse import bass_utils

S = 32.0
M = 0.5
EPS = 1e-07
B = 4096
C = 10000
NCORES = 8
R = B // NCORES  # rows per core = 512
P = 128  # partitions
NT = R // P  # row tiles per core = 4
NK = 2  # column chunks per row tile
F = C // NK  # columns per chunk = 5000

X_DT = mybir.dt.float16  # storage dtype for cls_score on device
X_NP = np.float16

COS_M = float(np.cos(M))
SIN_M = float(np.sin(M))
TAN_M = float(np.tan(M))

f32 = mybir.dt.float32
i32 = mybir.dt.int32

_NC_CACHE = {}


def _build_nc(n_iters: int = 1):
    nc = bacc.Bacc(
        "TRN2",
        target_bir_lowering=False,
        debug=False,
        num_devices=NCORES,
    )

    # x is declared flat [R*C, 1] so the same tensor serves both the row-tile
    # DMA view and the flat element-gather view for indirect DMA.
    x_h = nc.dram_tensor("x", [R * C, 1], X_DT, kind="ExternalInput")
    lab_h = nc.dram_tensor("lab", [NT, P, 1], i32, kind="ExternalInput")
    out_h = nc.dram_tensor("out", [1, 1], f32, kind="ExternalOutput")

    x_flat = x_h.ap()  # [R*C, 1] for the gather
    x_rows = x_h.ap().rearrange("(j p c) o -> j p (c o)", j=NT, p=P, c=C)

    with tile.TileContext(nc) as tc, ExitStack() as ctx:
        sing = ctx.enter_context(tc.tile_pool(name="sing", bufs=2))
        xin = ctx.enter_context(tc.tile_pool(name="xin", bufs=3))
        dump = ctx.enter_context(tc.tile_pool(name="dump", bufs=2))
        psum = ctx.enter_context(tc.tile_pool(name="psum", bufs=1, space="PSUM"))

        for _ in range(n_iters):
            _emit_iter(nc, tc, sing, xin, dump, psum, x_h, lab_h, out_h, x_flat, x_rows)

    nc.compile()
    return nc


def _emit_iter(nc, tc, sing, xin, dump, psum, x_h, lab_h, out_h, x_flat, x_rows):
    if True:
        # ---- per-row target gather: offsets = row_idx*C + label ----
        lab_t = sing.tile([P, NT], i32)
        for j in range(NT):
            nc.sync.dma_start(out=lab_t[:, j : j + 1], in_=lab_h.ap()[j])

        iota_t = sing.tile([P, 1], i32)
        nc.gpsimd.iota(iota_t[:], pattern=[[0, 1]], base=0, channel_multiplier=C)

        off_t = sing.tile([P, NT], i32)
        for j in range(NT):
            # off = (lab + j*P*C) + iota  (iota = partition_idx * C)
            nc.vector.scalar_tensor_tensor(
                out=off_t[:, j : j + 1],
                in0=lab_t[:, j : j + 1],
                scalar=j * P * C,
                in1=iota_t[:],
                op0=mybir.AluOpType.add,
                op1=mybir.AluOpType.add,
            )

        tgt_raw = sing.tile([P, NT], X_DT)
        for j in range(NT):
            nc.gpsimd.indirect_dma_start(
                out=tgt_raw[:, j : j + 1],
                out_offset=None,
                in_=x_flat,
                in_offset=bass.IndirectOffsetOnAxis(ap=off_t[:, j : j + 1], axis=0),
            )

        tgt = sing.tile([P, NT], f32)
        nc.vector.tensor_copy(out=tgt[:], in_=tgt_raw[:])

        # ---- numerator path ----
        # t = clip(tgt, -1+eps, 1-eps)
        t_cl = sing.tile([P, NT], f32)
        nc.vector.tensor_scalar(
            out=t_cl[:],
            in0=tgt[:],
            scalar1=-1.0 + EPS,
            scalar2=1.0 - EPS,
            op0=mybir.AluOpType.max,
            op1=mybir.AluOpType.min,
        )
        # mt2 = -t^2
        mt2 = sing.tile([P, NT], f32)
        nc.vector.scalar_tensor_tensor(
            out=mt2[:],
            in0=t_cl[:],
            scalar=-1.0,
            in1=t_cl[:],
            op0=mybir.AluOpType.mult,
            op1=mybir.AluOpType.mult,
        )
        # lnq = ln(1 - t^2)
        lnq = sing.tile([P, NT], f32)
        nc.scalar.activation(lnq[:], mt2[:], mybir.ActivationFunctionType.Ln, bias=1.0)
        # rt = sqrt(1-t^2) = exp(0.5*lnq)
        rt = sing.tile([P, NT], f32)
        nc.scalar.activation(
            rt[:], lnq[:], mybir.ActivationFunctionType.Exp, scale=0.5
        )
        # pre = t - tan(M)*rt ; num = S*cos(M)*pre
        pre = sing.tile([P, NT], f32)
        nc.vector.scalar_tensor_tensor(
            out=pre[:],
            in0=rt[:],
            scalar=-TAN_M,
            in1=t_cl[:],
            op0=mybir.AluOpType.mult,
            op1=mybir.AluOpType.add,
        )
        num = sing.tile([P, NT], f32)
        nc.vector.tensor_scalar_mul(num[:], pre[:], S * COS_M)
        # expnum = exp(num); expst = exp(S*t)
        expnum = sing.tile([P, NT], f32)
        nc.scalar.activation(expnum[:], num[:], mybir.ActivationFunctionType.Exp)
        expst = sing.tile([P, NT], f32)
        nc.scalar.activation(
            expst[:], t_cl[:], mybir.ActivationFunctionType.Exp, scale=S
        )

        # ---- main pass: exp(S*x) row-sums via ACT accumulate ----
        acc = sing.tile([P, NT * NK], f32)
        for j in range(NT):
            for k in range(NK):
                x_t = xin.tile([P, F], X_DT)
                nc.sync.dma_start(out=x_t[:], in_=x_rows[j][:, k * F : (k + 1) * F])
                e_t = dump.tile([P, F], X_DT, tag="edump")
                idx = j * NK + k
                nc.scalar.activation(
                    e_t[:],
                    x_t[:],
                    mybir.ActivationFunctionType.Exp,
                    scale=S,
                    accum_out=acc[:, idx : idx + 1],
                )

        # rs[p, j] = sum_k acc[p, j*NK+k]
        rs = sing.tile([P, NT], f32)
        acc_v = acc[:].rearrange("p (j k) -> p j k", k=NK)
        nc.vector.tensor_reduce(
            out=rs[:], in_=acc_v, axis=mybir.AxisListType.X, op=mybir.AluOpType.add
        )

        # denom = expnum + (rs - expst)
        den = sing.tile([P, NT], f32)
        nc.vector.scalar_tensor_tensor(
            out=den[:],
            in0=expst[:],
            scalar=-1.0,
            in1=rs[:],
            op0=mybir.AluOpType.mult,
            op1=mybir.AluOpType.add,
        )
        nc.vector.tensor_add(den[:], den[:], expnum[:])

        lnden = sing.tile([P, NT], f32)
        nc.scalar.activation(lnden[:], den[:], mybir.ActivationFunctionType.Ln)

        L = sing.tile([P, NT], f32)
        nc.vector.tensor_sub(L[:], num[:], lnden[:])

        Lr = sing.tile([P, 1], f32)
        nc.vector.tensor_reduce(
            out=Lr[:], in_=L[:], axis=mybir.AxisListType.X, op=mybir.AluOpType.add
        )

        # partial = sum_p Lr[p] * (-1/B)  via matmul against scaled ones
        ones = sing.tile([P, 1], f32)
        nc.vector.memset(ones[:], -1.0 / B)
        pt = psum.tile([1, 1], f32)
        nc.tensor.matmul(out=pt[:], lhsT=Lr[:], rhs=ones[:], start=True, stop=True)

        res_t = sing.tile([1, 1], f32)
        nc.vector.tensor_copy(out=res_t[:], in_=pt[:])
        nc.sync.dma_start(out=out_h.ap(), in_=res_t[:])


def _get_nc():
    if "nc" not in _NC_CACHE:
        _NC_CACHE["nc"] = _build_nc()
    return _NC_CACHE["nc"]


def kernel(cls_score: np.ndarray, label: np.ndarray, **run_kwargs) -> np.ndarray:
    cls_score = np.asarray(cls_score)
    label = np.asarray(label)
    assert cls_score.shape == (B, C), cls_score.shape

    nc = _get_nc()

    x16 = cls_score.astype(X_NP)
    lab32 = label.astype(np.int32)
    in_maps = []
    for i in range(NCORES):
        in_maps.append(
            {
                "x": np.ascontiguousarray(x16[i * R : (i + 1) * R]).reshape(R * C, 1),
                "lab": np.ascontiguousarray(lab32[i * R : (i + 1) * R]).reshape(
                    NT, P, 1
                ),
            }
        )

    res = bass_utils.run_bass_kernel_spmd(
        nc, in_maps, core_ids=list(range(NCORES)), **run_kwargs
    )
    partials = [np.asarray(r["out"]).reshape(()) for r in res.results]
    out = np.float32(np.sum(np.stack(partials), dtype=np.float64))
    if run_kwargs.get("trace"):
        return out, res
    return out
